# revision 38
# baseline (speedup 1.0000x reference)
"""NVFP4-style activation quantizer on 8 TRN2 NeuronCores (raw bass).

Reference semantics (per 16-element block, fp32):
    s_t  = max|x| / (6*448)                      (global, needs all-reduce)
    m_b  = max|x| over block
    inv  = 6 / (m_b / s_t)
    s_b  = fp8_e4m3_roundtrip(inv)   (the 0/inf guard is dead code for this
                                      input: inv >= 6/2688 = 2.23e-3 > 2^-10)
    out  = sign(x) * fp4_121(|x|/s_t * s_b) / s_b * s_t

All-16-bit quantize chain (measured rel_l2 vs reference: 1.05e-2, well
under the 2e-2 gate).  The fp4_121 magic-add works in fp16: the grid
step of the 1-2-1 code is ulp16(768 * max(2^e(y),1)), so

    y16 = x16 * c16                  (fp16 TT, 2x mode w/ dense c16)
    p   = bits16(y) & 0x7C00         (int16 TS, 4x mode)
    Bb  = max(p + 0x2600, 0x6200)    (int16 TS, 4x)  -> bits of 768*2^k
    t   = y + B                      (fp16 TT, 2x; internal fp32, RNE out)
    nq  = B - t                      (fp16 TT, 2x; = -fp4_121(y), exact)
    o   = nq * nic                   (TT vs fp16-broadcast nic, 1x, fp32 out)

~3.3 DVE cycles/element vs ~6 for the fp32 chain.  ScalarE feeds it:
per-tile fp32->fp16 conversion of x, materialization of dense c16 for
the cached tiles (the 2x mode needs unit-stride operands), and both
reciprocal families of the scale chain (spline Reciprocal, ~1e-7, used
off the banned-wrapper path; in-place over the block-max buffer).

HBM traffic: 1.5 reads + 1 write of x.  Tiles 0..TC-1 are cached in
SBUF as fp16 during pass A; tiles TC.. are re-read in pass B (their
y-multiply reads the per-block c16 broadcast at 1x, which sidesteps
the ScalarE-materialization critical path).  Tiles TC..TC+3 prefetch
and convert inside the AllReduce dead window.  GPSIMD runs only the
pre-warmed AllReduce.
"""

import numpy as np

FULL_SHAPE = (4, 4096, 4096)
N_CORES = 8
P = 128
TOTAL = 4 * 4096 * 4096
L = TOTAL // (N_CORES * P)   # 65536 elements per partition per core
NBLK = L // 16

F = 2048
T = L // F                   # 32 tiles
TC = 16                      # tiles cached as fp16 during pass A
NQ = 4                       # scale-chain quarters
FBLK = F // 16
QBLK = NBLK // NQ
TQ = T // NQ                 # tiles per quarter (quarter q covers 8 tiles)
N_XA = 4
N_CF = 3
N_OB = 2
EARLY_RR = 4                 # re-reads prefetched during the AR window

H_EXPMASK = 0x7C00
H_MAGIC_ADD = 0x2600
H_MAGIC_MIN = 0x6200

# scale-chain chunks (in blocks): a micro first chunk lets the first
# cfull/y start ~7us earlier after the AllReduce
CHUNKS = [128, 896, 1024, 1024, 1024]
CH_START = [sum(CHUNKS[:i]) for i in range(len(CHUNKS))]
NCH = len(CHUNKS)


def chunk_of_tile(t):
    b = t * FBLK
    for c in range(NCH):
        if CH_START[c] <= b < CH_START[c] + CHUNKS[c]:
            return c
    raise ValueError(t)


CH_FIRST_TILE = {}
for _t in range(T - 1, -1, -1):
    CH_FIRST_TILE[chunk_of_tile(_t)] = _t
CH_GATE = {v: k for k, v in CH_FIRST_TILE.items()}  # first tile -> chunk


def _plan_xa():
    """Order of DMAs into the xa slots; returns per-tile (slot, sem count)
    and the previous tile in the same slot (whose consumers gate reuse)."""
    order = list(range(T)) + list(range(TC, T))   # pass A, then re-reads
    count = [0] * N_XA
    need = {}
    prev = {}
    last = [None] * N_XA
    for i, t in enumerate(order):
        s = t % N_XA
        count[s] += 1
        key = (t, i >= T)
        need[key] = (s, 16 * count[s])
        prev[key] = last[s]
        last[s] = key
    return need, prev


XA_NEED, XA_PREV = _plan_xa()


def build_nc(n_cores=N_CORES):
    from contextlib import ExitStack

    import concourse.bass as bass
    from concourse import mybir

    f32 = mybir.dt.float32
    f16 = mybir.dt.float16
    i16 = mybir.dt.int16
    f8 = mybir.dt.float8e4

    nc = bass.Bass(num_devices=n_cores, debug=False)
    x_ext = nc.declare_dram_parameter("x", [P, L], f32, isOutput=False)
    out_ext = nc.declare_dram_parameter("out", [P, L], f32, isOutput=True)
    cc_in = nc.dram_tensor("cc_in", [1, 128], f32)
    cc_out = nc.dram_tensor("cc_out", [1, 128], f32, addr_space="Shared")
    cc_warm_in = nc.dram_tensor("cc_warm_in", [1, 128], f32)
    cc_warm_out = nc.dram_tensor("cc_warm_out", [1, 128], f32,
                                 addr_space="Shared")

    def act_reciprocal(act, out, in_):
        return act.add_instruction(
            mybir.InstActivation(
                name=act.bass.get_next_instruction_name(),
                func=mybir.ActivationFunctionType.Reciprocal,
                ins=[
                    act.lower_ap(in_),
                    mybir.ImmediateValue(dtype=f32, value=0.0),
                    mybir.ImmediateValue(dtype=f32, value=1.0),
                    mybir.ImmediateValue(dtype=f32, value=0.0),
                ],
                outs=[act.lower_ap(out)],
            )
        )

    with ExitStack() as ctx:
        def sem(name):
            return ctx.enter_context(nc.semaphore(name))

        def sbuf(name, shape, dt=f32):
            return ctx.enter_context(nc.sbuf_tensor(name, shape, dt))

        s_xa = [sem(f"s_xa{i}") for i in range(N_XA)]
        s_ob = [sem(f"s_ob{i}") for i in range(N_OB)]
        s_cdma = sem("s_cdma")
        s_dve = sem("s_dve")
        s_act = sem("s_act")     # ACT fp16 converts (+1, in tile order)
        s_cf = sem("s_cf")       # ACT cfull materializations (+1)
        s_acr = sem("s_acr")     # ACT reciprocals (+1)
        s_cc = sem("s_cc")
        s_pool = sem("s_pool")
        s_warm = sem("s_warm")

        # fp16 tile cache used as a ring: pass A fills slots 0..TC-1 with
        # tiles 0..TC-1; in pass B, re-read tile TC+k converts into slot k
        # once y(k) has consumed it.
        xh = sbuf("xh", [P, TC * F], f16)
        xa = [sbuf(f"xa{i}", [P, F]) for i in range(N_XA)]
        y16 = [sbuf(f"y16_{i}", [P, F], f16) for i in range(2)]
        pb16 = [sbuf(f"pb16_{i}", [P, F], i16) for i in range(2)]
        t16 = [sbuf(f"t16_{i}", [P, F], f16) for i in range(2)]
        nq16 = [sbuf(f"nq16_{i}", [P, F], f16) for i in range(2)]
        cfull = [sbuf(f"cfull{i}", [P, F], f16) for i in range(N_CF)]
        ob = [sbuf(f"ob{i}", [P, F]) for i in range(N_OB)]
        m_t = sbuf("m_t", [P, NBLK])     # blockmax -> 1/m (in place) -> s_b
        rs2 = [sbuf(f"rs2_{i}", [P, QBLK]) for i in range(2)]
        f8_t = sbuf("f8_t", [P, QBLK], f8)
        c16_t = sbuf("c16_t", [P, NBLK], f16)
        nic16_t = sbuf("nic16_t", [P, NBLK], f16)
        gall_t = sbuf("gall_t", [P, 128])
        mxq_t = sbuf("mxq_t", [P, NQ])
        mx_t = sbuf("mx_t", [P, 1])
        g128_t = sbuf("g128_t", [P, 1])
        st_t = sbuf("st_t", [P, 1])
        rt_t = sbuf("rt_t", [P, 1])
        k6_t = sbuf("k6_t", [P, 1])
        nst_t = sbuf("nst_t", [P, 1])

        dveA = [0] * T
        K_mxq = [0] * NQ
        tag_y = [0] * T
        tag_nq = [0] * T
        tag_o = [0] * T
        K_mx = [0]
        K_sb = [0] * NCH
        K_c = [0] * NCH
        K_nic = [0] * NCH

        def b3(ap):
            return ap.rearrange("p (b s) -> p b s", s=16)

        def qs(q):
            return slice(q * QBLK, (q + 1) * QBLK)

        def conv_done(t):
            """s_act value after conv(t): convs run in tile order
            0..TC-1 (pass A) then TC..T-1."""
            return t + 1

        with nc.Block() as block:

            @block.vector
            def _(dve):
                cnt = 0

                def tag(ins):
                    nonlocal cnt
                    ins.then_inc(s_dve)
                    cnt += 1
                    return cnt

                # ---- pass A: per-block abs max ----
                for t in range(T):
                    dve.wait_ge(s_xa[XA_NEED[(t, False)][0]],
                                XA_NEED[(t, False)][1])
                    dveA[t] = tag(dve.tensor_reduce(
                        out=m_t[:, t * FBLK:(t + 1) * FBLK],
                        in_=b3(xa[t % N_XA][:]),
                        axis=mybir.AxisListType.X,
                        op=mybir.AluOpType.max,
                        apply_absolute_value=True,
                    ))
                    if (t + 1) % TQ == 0:
                        # partial max of this m-quarter, so ACT's in-place
                        # 1/m can start before the global reduce
                        q = t // TQ
                        dve.wait_ge(s_dve, dveA[t])
                        K_mxq[q] = tag(dve.tensor_reduce(
                            out=mxq_t[:, q:q + 1], in_=m_t[:, qs(q)],
                            axis=mybir.AxisListType.X,
                            op=mybir.AluOpType.max,
                        ))
                dve.wait_ge(s_dve, K_mxq[NQ - 1])
                K_mx[0] = tag(dve.tensor_reduce(
                    out=mx_t[:], in_=mxq_t[:], axis=mybir.AxisListType.X,
                    op=mybir.AluOpType.max,
                ))

                # ---- global scalars (post-AllReduce) ----
                dve.wait_ge(s_cdma, 32)
                k = tag(dve.tensor_reduce(
                    out=g128_t[:], in_=gall_t[:], axis=mybir.AxisListType.X,
                    op=mybir.AluOpType.max))
                dve.wait_ge(s_dve, k)
                k_st = tag(dve.tensor_scalar(
                    st_t[:], g128_t[:], 1.0 / 2688.0, None,
                    op0=mybir.AluOpType.mult))
                dve.wait_ge(s_dve, k_st)
                k_rt = tag(dve.reciprocal(rt_t[:], st_t[:]))
                k_k6 = tag(dve.tensor_scalar(
                    k6_t[:], st_t[:], 6.0, None, op0=mybir.AluOpType.mult))
                k_nst = tag(dve.tensor_scalar(
                    nst_t[:], st_t[:], -1.0, None, op0=mybir.AluOpType.mult))
                dve.wait_ge(s_dve, k_nst)

                # ---- per-block scales, in CHUNKS (rm already in m_t) ----
                # chunk c needs rm coverage: chunk->rm-quarter wait
                rmq = [1, 1, 2, 3, 4]
                for c in range(NCH):
                    cs = slice(CH_START[c], CH_START[c] + CHUNKS[c])
                    n = CHUNKS[c]
                    dve.wait_ge(s_acr, rmq[c])
                    k_f8 = tag(dve.tensor_scalar(
                        f8_t[:, 0:n], m_t[:, cs], k6_t[:], None,
                        op0=mybir.AluOpType.mult))
                    dve.wait_ge(s_dve, k_f8)
                    K_sb[c] = tag(dve.tensor_copy(m_t[:, cs], f8_t[:, 0:n]))
                    dve.wait_ge(s_dve, K_sb[c])
                    K_c[c] = tag(dve.tensor_scalar(
                        c16_t[:, cs], m_t[:, cs], rt_t[:], None,
                        op0=mybir.AluOpType.mult))
                    if c >= 1:
                        pc = c - 1
                        pcs = slice(CH_START[pc], CH_START[pc] + CHUNKS[pc])
                        dve.wait_ge(s_acr, NQ + pc + 1)
                        K_nic[pc] = tag(dve.tensor_scalar(
                            nic16_t[:, pcs], rs2[pc % 2][:, 0:CHUNKS[pc]],
                            nst_t[:], None, op0=mybir.AluOpType.mult))
                dve.wait_ge(s_acr, NQ + NCH)
                lc = NCH - 1
                lcs = slice(CH_START[lc], CH_START[lc] + CHUNKS[lc])
                K_nic[lc] = tag(dve.tensor_scalar(
                    nic16_t[:, lcs], rs2[lc % 2][:, 0:CHUNKS[lc]],
                    nst_t[:], None, op0=mybir.AluOpType.mult))

                # ---- pass B: 16-bit quantize chain, pairs of tiles ----
                tag_pb = [0] * T
                tag_t = [0] * T
                for tp in range(0, T, 2):
                    pair = (tp, tp + 1)
                    for t in pair:
                        if t >= 2:
                            dve.wait_ge(s_dve, tag_o[t - 2])
                        if t in CH_GATE:
                            dve.wait_ge(s_dve, K_c[CH_GATE[t]])
                        dve.wait_ge(s_cf, t + 1)
                        if t >= TC:
                            dve.wait_ge(s_act, conv_done(t))
                        sl = t % TC
                        tag_y[t] = tag(dve.tensor_tensor(
                            y16[t % 2][:], xh[:, sl * F:(sl + 1) * F],
                            cfull[t % N_CF][:],
                            op=mybir.AluOpType.mult))
                    for t in pair:
                        dve.wait_ge(s_dve, tag_y[t])
                        tag_pb[t] = tag(dve.tensor_scalar(
                            t16[t % 2][:].bitcast(i16),
                            y16[t % 2][:].bitcast(i16),
                            H_EXPMASK, None,
                            op0=mybir.AluOpType.bitwise_and))
                    for t in pair:
                        dve.wait_ge(s_dve, tag_pb[t])
                        tag_pb[t] = tag(dve.tensor_scalar(
                            pb16[t % 2][:], t16[t % 2][:].bitcast(i16),
                            H_MAGIC_ADD, H_MAGIC_MIN,
                            op0=mybir.AluOpType.add,
                            op1=mybir.AluOpType.max))
                    for t in pair:
                        dve.wait_ge(s_dve, tag_pb[t])
                        tag_t[t] = tag(dve.tensor_tensor(
                            t16[t % 2][:], y16[t % 2][:],
                            pb16[t % 2][:].bitcast(f16),
                            op=mybir.AluOpType.add))
                    for t in pair:
                        dve.wait_ge(s_dve, tag_t[t])
                        tag_nq[t] = tag(dve.tensor_tensor(
                            nq16[t % 2][:], pb16[t % 2][:].bitcast(f16),
                            t16[t % 2][:], op=mybir.AluOpType.subtract))
                    for t in pair:
                        bsl = slice(t * FBLK, (t + 1) * FBLK)
                        dve.wait_ge(s_dve, tag_nq[t])
                        if t >= 2:
                            dve.wait_ge(s_ob[t % N_OB],
                                        16 * ((t - 2) // 2 + 1))
                        if t in CH_GATE:
                            dve.wait_ge(s_dve, K_nic[CH_GATE[t]])
                        tag_o[t] = tag(dve.tensor_tensor(
                            b3(ob[t % N_OB][:]), b3(nq16[t % 2][:]),
                            nic16_t[:, bsl].unsqueeze(-1).broadcast_to(
                                [P, FBLK, 16]),
                            op=mybir.AluOpType.mult))

            @block.scalar
            def _(act):
                # pass A: odd input DMAs issue from this queue (doubles
                # HWDGE issue bandwidth), interleaved with the fp16
                # conversions of the cached tiles
                for t in range(T):
                    if t % 2 == 1 and t >= 3:
                        prev = XA_PREV[(t, False)]
                        if prev is not None:
                            pt = prev[0]
                            act.wait_ge(s_dve, dveA[pt])
                            if pt < TC:
                                act.wait_ge(s_act, conv_done(pt))
                        act.dma_start(
                            out=xa[t % N_XA][:, :],
                            in_=x_ext[:, t * F:(t + 1) * F],
                        ).then_inc(s_xa[t % N_XA], 16)
                    c = t - 1
                    if 0 <= c < TC:
                        act.wait_ge(s_xa[XA_NEED[(c, False)][0]],
                                    XA_NEED[(c, False)][1])
                        act.activation(
                            xh[:, c * F:(c + 1) * F], xa[c % N_XA][:],
                            mybir.ActivationFunctionType.Copy,
                        ).then_inc(s_act)
                # rm = 1/m, in place, per quarter (AR-independent; gated
                # on the quarter's partial max so m is fully consumed)
                for q in range(NQ):
                    act.wait_ge(s_dve, K_mxq[q])
                    act_reciprocal(act, m_t[:, qs(q)],
                                   m_t[:, qs(q)]).then_inc(s_acr)
                # rs = 1/s_b per chunk
                for c in range(NCH):
                    cs = slice(CH_START[c], CH_START[c] + CHUNKS[c])
                    act.wait_ge(s_dve, K_sb[c])
                    if c >= 2:
                        act.wait_ge(s_dve, K_nic[c - 2])
                    act_reciprocal(act, rs2[c % 2][:, 0:CHUNKS[c]],
                                   m_t[:, cs]).then_inc(s_acr)

                # pass B: cfull per tile; re-read conversions (into the
                # ring slot their predecessor vacated) run 3 tiles ahead
                # of their consumer so the DVE never waits on them
                def conv(t):
                    act.wait_ge(s_xa[XA_NEED[(t, True)][0]],
                                XA_NEED[(t, True)][1])
                    act.wait_ge(s_dve, tag_y[t - TC])
                    sl = t % TC
                    act.activation(
                        xh[:, sl * F:(sl + 1) * F], xa[t % N_XA][:],
                        mybir.ActivationFunctionType.Copy,
                    ).then_inc(s_act)

                for t in range(T):
                    bsl = slice(t * FBLK, (t + 1) * FBLK)
                    if t >= N_CF:
                        act.wait_ge(s_dve, tag_y[t - N_CF])
                    act.wait_ge(s_dve, K_c[chunk_of_tile(t)])
                    act.activation(
                        b3(cfull[t % N_CF][:]),
                        c16_t[:, bsl].unsqueeze(-1).broadcast_to(
                            [P, FBLK, 16]),
                        mybir.ActivationFunctionType.Copy,
                    ).then_inc(s_cf)
                    r = t + 3
                    if TC <= r < T:
                        conv(r)

            @block.gpsimd
            def _(pool):
                pool.memset(gall_t[0:1, :], 0.0).then_inc(s_pool)
                pool.wait_ge(s_pool, 1)
                pool.dma_start(out=cc_warm_in[:, :],
                               in_=gall_t[0:1, :]).then_inc(s_warm, 16)
                pool.wait_ge(s_warm, 16)
                pool.collective_compute(
                    "AllReduce",
                    mybir.AluOpType.max,
                    replica_groups=[list(range(n_cores))],
                    ins=[cc_warm_in.ap().opt()],
                    outs=[cc_warm_out.ap().opt()],
                ).then_inc(s_cc)
                pool.wait_ge(s_cdma, 16)
                pool.collective_compute(
                    "AllReduce",
                    mybir.AluOpType.max,
                    replica_groups=[list(range(n_cores))],
                    ins=[cc_in.ap().opt()],
                    outs=[cc_out.ap().opt()],
                ).then_inc(s_cc)
                # pass-B re-read DMAs (tiles TC+EARLY_RR..), issued from
                # this otherwise-idle queue as their xa slot frees
                for r in range(TC + EARLY_RR, T):
                    prev = XA_PREV[(r, True)]
                    pt = prev[0]
                    pool.wait_ge(s_act, conv_done(pt))
                    pool.dma_start(
                        out=xa[r % N_XA][:, :],
                        in_=x_ext[:, r * F:(r + 1) * F],
                    ).then_inc(s_xa[r % N_XA], 16)

            @block.sync
            def _(sync):
                def rr_wait(t):
                    prev = XA_PREV[(t, True)]
                    if prev is None:
                        return
                    pt, was_rr = prev
                    if not was_rr:
                        sync.wait_ge(s_dve, dveA[pt])
                        if pt < TC:
                            sync.wait_ge(s_act, conv_done(pt))
                    else:
                        sync.wait_ge(s_act, conv_done(pt))

                # pass A input DMAs (tiles 0,1,2 primed here, then even
                # tiles; odd tiles >=3 issue from the ACT queue)
                for t in [0, 1] + list(range(2, T, 2)):
                    prev = XA_PREV[(t, False)]
                    if prev is not None:
                        pt = prev[0]
                        sync.wait_ge(s_dve, dveA[pt])
                        if pt < TC:
                            sync.wait_ge(s_act, conv_done(pt))
                    sync.dma_start(
                        out=xa[t % N_XA][:, :],
                        in_=x_ext[:, t * F:(t + 1) * F],
                    ).then_inc(s_xa[t % N_XA], 16)
                # early re-read prefetches (overlap the AllReduce window)
                for t in range(TC, TC + EARLY_RR):
                    rr_wait(t)
                    sync.dma_start(
                        out=xa[t % N_XA][:, :],
                        in_=x_ext[:, t * F:(t + 1) * F],
                    ).then_inc(s_xa[t % N_XA], 16)
                # collective staging
                sync.wait_ge(s_dve, K_mx[0])
                sync.dma_start(out=cc_in[:, :], in_=mx_t[:, :]).then_inc(
                    s_cdma, 16)
                sync.wait_ge(s_cc, 2)
                sync.dma_start(
                    out=gall_t[:, :],
                    in_=cc_out.ap().broadcast_to([P, 128]),
                ).then_inc(s_cdma, 16)
                # pass B: out DMAs only (re-reads go via the idle GPSIMD
                # queue so this queue stays under the output bandwidth)
                for t in range(T):
                    sync.wait_ge(s_dve, tag_o[t])
                    sync.dma_start(
                        out=out_ext[:, t * F:(t + 1) * F],
                        in_=ob[t % N_OB][:, :],
                    ).then_inc(s_ob[t % N_OB], 16)
                for i in range(N_OB):
                    uses = len([t for t in range(T) if t % N_OB == i])
                    sync.wait_ge(s_ob[i], 16 * uses)

    return nc


_CACHE = {}


def _get_nc():
    if "nc" not in _CACHE:
        _CACHE["nc"] = build_nc()
    return _CACHE["nc"]


def kernel(x: np.ndarray) -> np.ndarray:
    from concourse.bass_utils import run_bass_kernel_spmd

    x = np.asarray(x, dtype=np.float32)
    assert x.shape == FULL_SHAPE
    shards = x.reshape(N_CORES, P, L)
    in_maps = [{"x": np.ascontiguousarray(shards[i])} for i in range(N_CORES)]
    nc = _get_nc()
    res = run_bass_kernel_spmd(nc, in_maps, core_ids=list(range(N_CORES)))
    out = np.stack([r["out"] for r in res.results], axis=0)
    return out.reshape(FULL_SHAPE)
